# revision 1
# baseline (speedup 1.0000x reference)
"""Bottom-Up Hidden Tree Markov Model upward pass on 8 Trainium2 NeuronCores.

Problem: complete 8-ary forest (2 trees x 299593 nodes, depth 6), C=8 hidden
states, 32 symbols, 16 independent generative models. Output: per-tree
log-likelihood (2, 16).

Sharding: core = (tree, quarter-of-tree). Each core runs the full upward pass
over its quarter (2 complete depth-1 subtrees): 65536 leaves -> 8192 -> 1024
-> 128 -> 16 -> 2 level-1 betas. Host combines the 8 level-1 betas per tree
with the tiny root step.

Key algebraic restructurings (device does all O(N) work):
  - Leaf betas depend only on (position l, symbol s): they collapse into a
    256-row table; the level-6 einsum folds into T6[(l,s),(i,g)] so the whole
    leaf level becomes one-hot(symbol) matmuls.
  - Leaf log-nu contributions become histogram-counts x log-table (counts fall
    out of the one-hot generation for free via accum_out).
  - Interior levels: blocked matmuls with block-diagonal-over-g weights
    W_l[(j,g),(i,g)]; per-node normalize via sel/broadcast matmuls; log-nu via
    ScalarE Ln with free accumulation.
Partition packing everywhere: p = i*16 + g  (i = hidden state, g = generator).
"""
import os
import sys
import tempfile

import numpy as np

if '/opt/trn_rl_repo' not in sys.path:
    sys.path.insert(0, '/opt/trn_rl_repo')

import ml_dtypes

BF16 = ml_dtypes.bfloat16

K, DEPTH, NTREE, C, MSYM, NGEN = 8, 6, 2, 8, 32, 16
STARTS = [(K ** d - 1) // (K - 1) for d in range(DEPTH + 2)]
NT = STARTS[DEPTH + 1]          # 299593 nodes per tree
CG = C * NGEN                   # 128
NQ = 4                          # quarters per tree
LEAVES_Q = (K ** DEPTH) // NQ   # 65536 leaves per core
XP_LEN = 8192 + 1024 + 128 + 16 + 2   # interior-node symbols per core
XP_PAD = 9376
# per-level (parents U, chunks, xp offset)
LEVELS = [
    (8192, 16, 0),
    (1024, 2, 8192),
    (128, 1, 9216),
    (16, 1, 9344),
    (2, 1, 9360),
]
N_LL_SLOTS = 16 + 2 + 1 + 1 + 1 + 1   # per-chunk ll partials + leaf slot


def _softmax64(x, axis):
    x = np.asarray(x, np.float64)
    e = np.exp(x - x.max(axis=axis, keepdims=True))
    return e / e.sum(axis=axis, keepdims=True)


def _build_tables(A, B, Pi, SP):
    """Small O(params) tables, f64 on host. Returns dict of np arrays."""
    smA = _softmax64(A, 0)            # (C,C,K,G) over parent state i
    smB = _softmax64(B, 1)            # (C,M,G) over symbols
    smPi = _softmax64(Pi, 0)          # (C,K,G)
    smSP = _softmax64(SP, 0)          # (K,G)
    Mmat = smSP[:, None, None, :] * np.transpose(smA, (2, 0, 1, 3))  # [l,i,j,g]
    pb = smPi[:, :, None, :] * smB[:, None, :, :]     # (j, l, s, g)
    nuL = pb.sum(0)                                    # (l, s, g)
    betaLeaf = pb / nuL[None]
    llLeaf = np.log(nuL)                               # (l, s, g)
    T6 = np.einsum('lijg,jlsg->lsig', Mmat, betaLeaf)  # (l,s,i,g)
    T6f = T6.reshape(K * MSYM, CG)                     # rows (l,s), cols (i,g)
    Wl = np.zeros((K, CG, CG))
    ii = np.arange(C)
    for l in range(K):
        for g in range(NGEN):
            Wl[l, ii[:, None] * NGEN + g, ii[None, :] * NGEN + g] = Mmat[l, :, :, g].T
    BT = np.transpose(smB, (1, 0, 2)).reshape(MSYM, CG)
    llLeaf_f = llLeaf.reshape(K * MSYM, NGEN)

    p = np.arange(CG)
    sel = (p[:, None] % NGEN == np.arange(NGEN)[None, :]).astype(np.float64)
    E16 = sel.T.copy()
    return {
        'T6a': T6f[:128].astype(BF16),
        'T6b': T6f[128:].astype(BF16),
        'Wt': np.concatenate([Wl[l] for l in range(K)], axis=1).astype(BF16),  # [128, 1024]
        'BTt': BT.astype(BF16),                       # [32, 128]
        'selt': sel.astype(BF16),                     # [128, 16]
        'E16t': E16.astype(BF16),                     # [16, 128]
        'E16x4': np.concatenate([np.vstack([E16, np.zeros((16, CG))]) for _ in range(4)], axis=0).astype(BF16),  # [128, 128]
        'svec': (np.arange(128) % MSYM).reshape(128, 1).astype(np.float32),
        'svec32': np.arange(32).reshape(32, 1).astype(np.float32),
        'llLA': llLeaf_f[:128].astype(np.float32),    # [128, 16]
        'llLB': llLeaf_f[128:].astype(np.float32),    # [128, 16]
    }, Mmat.astype(np.float32), np.asarray(smB, np.float32)


def _build_bass():
    import concourse.bass as bass
    import concourse.bacc as bacc
    import concourse.mybir as mybir
    from concourse import tile

    f32 = mybir.dt.float32
    bf16 = mybir.dt.bfloat16
    Alu = mybir.AluOpType
    Act = mybir.ActivationFunctionType

    nc = bacc.Bacc(None, target_bir_lowering=False)

    # inputs
    u8 = mybir.dt.uint8
    xs_d = nc.dram_tensor('xs', [K, LEAVES_Q // K], u8, kind='ExternalInput')
    xp_d = nc.dram_tensor('xp', [1, XP_PAD], u8, kind='ExternalInput')
    tab_specs = [
        ('svec', [128, 1], f32), ('svec32', [32, 1], f32),
        ('T6a', [128, 128], bf16), ('T6b', [128, 128], bf16),
        ('BTt', [32, 128], bf16),
        ('selt', [128, 16], bf16), ('E16t', [16, 128], bf16),
        ('E16x4', [128, 128], bf16), ('Wt', [128, 1024], bf16),
        ('llLA', [128, 16], f32), ('llLB', [128, 16], f32),
    ]
    tab_d = {n: nc.dram_tensor(n, s, d, kind='ExternalInput') for n, s, d in tab_specs}
    beta1_d = nc.dram_tensor('beta1', [128, 2], f32, kind='ExternalOutput')
    llsum_d = nc.dram_tensor('llsum', [16, 1], f32, kind='ExternalOutput')

    with tile.TileContext(nc) as tc:
        with (
            tc.tile_pool(name='const', bufs=1) as constp,
            tc.tile_pool(name='beta', bufs=1) as betap,
            tc.tile_pool(name='oh', bufs=3) as ohp,
            tc.tile_pool(name='ohp32', bufs=3) as ohp32p,
            tc.tile_pool(name='bl', bufs=6) as blp,
            tc.tile_pool(name='bxs', bufs=4) as bxsbp,
            tc.tile_pool(name='rr', bufs=3) as rrp,
            tc.tile_pool(name='bxs2', bufs=3) as bxs2p,
            tc.tile_pool(name='lnout', bufs=3) as lnp,
            tc.tile_pool(name='acc', bufs=1) as accp,
            tc.tile_pool(name='cntp', bufs=8) as cntp,
            tc.tile_pool(name='xsb', bufs=6) as xsbp,
            tc.tile_pool(name='ps_tb', bufs=2, space='PSUM') as ps_tb,
            tc.tile_pool(name='ps_bx', bufs=2, space='PSUM') as ps_bx,
            tc.tile_pool(name='ps_nu', bufs=2, space='PSUM') as ps_nu,
            tc.tile_pool(name='ps_rb', bufs=2, space='PSUM') as ps_rb,
        ):
            # load constant tables
            tab = {}
            for n, s, d in tab_specs:
                t = constp.tile(s, d, tag=n)
                nc.sync.dma_start(t[:], tab_d[n][:])
                tab[n] = t

            xpb_all = accp.tile([32, XP_PAD], u8, name='xpb_all', tag='xpb_all')

            beta_bufs = [
                betap.tile([128, 8192], bf16, name='b5', tag='b5'),
                betap.tile([128, 1024], bf16, name='b4', tag='b4'),
                betap.tile([128, 128], bf16, name='b3', tag='b3'),
                betap.tile([128, 16], bf16, name='b2', tag='b2'),
                betap.tile([128, 2], f32, name='b1', tag='b1'),
            ]
            llparts = accp.tile([16, N_LL_SLOTS], f32, name='llparts', tag='llparts')
            cnts = accp.tile([128, 8], f32, name='cnts', tag='cnts')
            llsum_sb = accp.tile([16, 1], f32, name='llsum', tag='llsum')
            cA = accp.tile([128, 1], f32, name='cA', tag='cA')
            cB = accp.tile([128, 1], f32, name='cB', tag='cB')

            slot = 0
            cnt_tiles = []
            bl_pend = []
            GRP = 4      # chunks per one-hot batch at level 6
            for lev, (U, nch, xpo) in enumerate(LEVELS):
                N = U // nch
                out_beta = beta_bufs[lev]
                oh_grp = {}
                for c in range(nch):
                    tb_ps = ps_tb.tile([128, N], f32, name='tb', tag='tb')
                    # Bx = B[:, x_parent] via one-hot matmul
                    if c % GRP == 0:
                        NPW = min(N * GRP, U - (c // GRP) * N * GRP)
                        po = xpo + (c // GRP) * N * GRP
                        xp_bcast = bass.AP(xp_d[:].tensor, po, [[0, 32], [1, NPW]])
                        nc.sync.dma_start(xpb_all[:, po: po + NPW], xp_bcast)
                        ohP_g = ohp32p.tile([32, NPW], bf16, name='ohP', tag='ohP')
                        sv = tab['svec32'][:]
                        svb = bass.AP(sv.tensor, sv.offset, [list(sv.ap[0]), [0, NPW]])
                        nc.vector.tensor_tensor(ohP_g[:], xpb_all[:, po: po + NPW],
                                                svb, Alu.is_equal)
                        oh_grp['P'] = ohP_g
                    po2 = (c % GRP) * N
                    bx_ps = ps_bx.tile([128, N], f32, name='bx', tag='bx')
                    nc.tensor.matmul(bx_ps[:], tab['BTt'][:], oh_grp['P'][:, po2:po2 + N], start=True, stop=True)
                    bx_sb = bxsbp.tile([128, N], bf16, name='bxs', tag='bxs')
                    nc.scalar.copy(bx_sb[:], bx_ps[:])
                    if lev == 0:
                        if c % GRP == 0:
                            g4 = c // GRP
                            NW = N * GRP
                            ohA_g = ohp.tile([128, NW], bf16, name='ohA', tag='ohA')
                            ohB_g = ohp.tile([128, NW], bf16, name='ohB', tag='ohB')
                            for goff, oh_t, cslot in ((0, ohA_g, g4), (4, ohB_g, 4 + g4)):
                                xsb_t = xsbp.tile([128, NW], u8, name='xsb', tag='xsb')
                                src_ap = bass.AP(xs_d[:].tensor, goff * (LEAVES_Q // K) + g4 * NW,
                                                 [[LEAVES_Q // K, 4], [0, 32], [1, NW]])
                                nc.sync.dma_start(xsb_t[:], src_ap)
                                cntc = cntp.tile([128, 1], f32, name='cntc', tag='cntc')
                                nc.vector.tensor_scalar(
                                    oh_t[:], xsb_t[:], tab['svec'][:], None,
                                    Alu.is_equal, Alu.add,
                                    accum_out=cntc[:])
                                nc.scalar.copy(cnts[:, cslot:cslot + 1], cntc[:])
                            oh_grp.update({'A': ohA_g, 'B': ohB_g})
                        co = (c % GRP) * N
                        nc.tensor.matmul(tb_ps[:], tab['T6a'][:], oh_grp['A'][:, co:co + N], start=True, stop=False)
                        nc.tensor.matmul(tb_ps[:], tab['T6b'][:], oh_grp['B'][:, co:co + N], start=False, stop=True)
                    else:
                        child = beta_bufs[lev - 1]
                        bview = child[:].rearrange('p (u l) -> p u l', l=K)
                        for l in range(K):
                            nc.tensor.matmul(
                                tb_ps[:], tab['Wt'][:, 128 * l:128 * (l + 1)],
                                bview[:, c * N:(c + 1) * N, l],
                                start=(l == 0), stop=(l == K - 1))
                    # bl, nu, ll, normalize; nu for 4 chunks packs into one PSUM
                    # tile at 32-partition offsets so one reciprocal serves all 4
                    bl_t = blp.tile([128, N], bf16, name='bl', tag='bl')
                    nc.vector.tensor_mul(bl_t[:], tb_ps[:], bx_sb[:])
                    if c % GRP == 0:
                        nu4_ps = ps_nu.tile([128, N], f32, name='nu4', tag='nu4')
                        nc.vector.memset(nu4_ps[:], 1.0)
                    poff = 32 * (c % GRP)
                    nu_sl = nu4_ps[poff:poff + 16, :]
                    nc.tensor.matmul(nu_sl, tab['selt'][:], bl_t[:], start=True, stop=True,
                                     tile_position=(0, poff))
                    ln_t = lnp.tile([16, N], f32, name='ln', tag='ln')
                    nc.scalar.activation(ln_t[:], nu_sl, Act.Ln,
                                         accum_out=llparts[:, slot:slot + 1])
                    bl_pend.append((bl_t, c, poff))
                    if c % GRP == GRP - 1 or c == nch - 1:
                        r4_t = rrp.tile([128, N], bf16, name='r4', tag='r4')
                        with nc.allow_low_precision(reason='bf16 recip broadcast validated in numpy'):
                            nc.vector.reciprocal(r4_t[:], nu4_ps[:])
                        for bl_p, cp, poffp in bl_pend:
                            rb_ps = ps_rb.tile([128, N], f32, name='rb', tag='rb')
                            nc.tensor.matmul(rb_ps[:], tab['E16x4'][poffp:poffp + 16, :],
                                             r4_t[poffp:poffp + 16, :], start=True, stop=True,
                                             tile_position=(poffp, 0))
                            nc.vector.tensor_mul(out_beta[:, cp * N:(cp + 1) * N], bl_p[:], rb_ps[:])
                        bl_pend = []
                    slot += 1

            # leaf ll from histogram counts
            nc.vector.reduce_sum(cA[:], cnts[:, 0:4], axis=mybir.AxisListType.X)
            nc.vector.reduce_sum(cB[:], cnts[:, 4:8], axis=mybir.AxisListType.X)
            llf_ps = ps_nu.tile([16, 1], f32, name='llf', tag='nu4')
            nc.tensor.matmul(llf_ps[:], tab['llLA'][:], cA[:], start=True, stop=False)
            nc.tensor.matmul(llf_ps[:], tab['llLB'][:], cB[:], start=False, stop=True)
            nc.scalar.copy(llparts[:, slot:slot + 1], llf_ps[:])

            nc.vector.reduce_sum(llsum_sb[:], llparts[:], axis=mybir.AxisListType.X)
            nc.sync.dma_start(llsum_d[:], llsum_sb[:])
            nc.sync.dma_start(beta1_d[:], beta_bufs[4][:])
    if not nc.is_finalized():
        nc.finalize()
    return nc


_BASS_CACHE = {}


def _get_bass():
    if 'nc' not in _BASS_CACHE:
        _BASS_CACHE['nc'] = _build_bass()
    return _BASS_CACHE['nc']


def kernel(**inputs):
    from concourse.bass_utils import run_bass_kernel_spmd

    A = np.asarray(inputs['A']); B = np.asarray(inputs['B'])
    Pi = np.asarray(inputs['Pi']); SP = np.asarray(inputs['SP'])
    x = np.asarray(inputs['x'])

    tables, Mmat, smB = _build_tables(A, B, Pi, SP)

    in_maps = []
    for t in range(NTREE):
        base = t * NT
        for q in range(NQ):
            s6 = base + STARTS[6] + q * LEAVES_Q
            xs = x[s6: s6 + LEAVES_Q].astype(np.uint8)
            xs_t = np.ascontiguousarray(xs.reshape(LEAVES_Q // K, K).T)  # [8, 8192]
            xp = np.zeros((1, XP_PAD), np.uint8)
            off = 0
            for d in range(5, 0, -1):
                n_d = K ** d
                s = base + STARTS[d] + q * (n_d // NQ)
                xp[0, off: off + n_d // NQ] = x[s: s + n_d // NQ].astype(np.uint8)
                off += n_d // NQ
            m = {'xs': xs_t, 'xp': xp}
            m.update(tables)
            in_maps.append(m)

    nc = _get_bass()
    global _LAST_IN_MAPS
    _LAST_IN_MAPS = in_maps
    res = run_bass_kernel_spmd(nc, in_maps, core_ids=list(range(8)))
    results = res.results

    out = np.zeros((NTREE, NGEN), np.float32)
    for t in range(NTREE):
        beta1 = np.zeros((K, C, NGEN), np.float32)
        for q in range(NQ):
            r = results[t * NQ + q]
            out[t] += r['llsum'][:, 0]
            # beta1 dram [128, 2]: column n is quarter-node n, packing p=i*16+g
            beta1[2 * q: 2 * q + 2] = r['beta1'].T.reshape(2, C, NGEN)
        tb = np.einsum('lijg,ljg->ig', Mmat, beta1)
        bl = tb * smB[:, x[t * NT]]
        nu = bl.sum(0)
        out[t] += np.log(nu).astype(np.float32)
    return out



# revision 4
# speedup vs baseline: 1.7547x; 1.7547x over previous
"""Bottom-Up Hidden Tree Markov Model upward pass on 8 Trainium2 NeuronCores.

Problem: complete 8-ary forest (2 trees x 299593 nodes, depth 6), C=8 hidden
states, 32 symbols, 16 independent generative models. Output: per-tree
log-likelihood (2, 16).

Sharding: core = (tree, quarter-of-tree). Each core runs the upward pass over
its quarter's two big levels (98.4% of all nodes): 65536 leaves -> 8192
level-5 betas (one fp8 DoubleRow matmul per 512 parents against the collapsed
leaf table T6, contracting all 256 (position, symbol) one-hot rows at once)
and 8192 -> 1024 level-4 t_betas (block-diagonal W matmuls). The host applies
the exact softmax recursion to the tiny tree top (levels 4..1 + root, 1.6% of
nodes) and the leaf log-nu histogram, both in float64.

Device pipeline per 512-parent chunk (engines balanced, dispatch-minimal):
  PE:   tb   = [T6a;T6b]^T [ohA;ohB]           (fp8 DoubleRow, PSUM f32)
  DVE:  bl   = tb * bx                          (bf16 out)
  PE:   nu_b = SR^T bl                          (per-g sums broadcast to all
                                                 128 partitions in one matmul)
  Act:  Ln(nu) with free-dim accumulation       -> per-chunk log-nu partials
  Pool: beta = bl / nu_b                        (divide, bf16 out)
Partition packing everywhere: p = i*16 + g  (i = hidden state, g = generator).

T6 and bx are carried in fp8_e4m3 scaled by 128 (one-hots exact, ~2^-4
relative quantization); the uniform 128^2 scale cancels in beta and is
subtracted from the accumulated log-nus on the host.
"""
import sys

import numpy as np

if '/opt/trn_rl_repo' not in sys.path:
    sys.path.insert(0, '/opt/trn_rl_repo')

import ml_dtypes

BF16 = ml_dtypes.bfloat16
F8 = ml_dtypes.float8_e4m3

K, DEPTH, NTREE, C, MSYM, NGEN = 8, 6, 2, 8, 32, 16
STARTS = [(K ** d - 1) // (K - 1) for d in range(DEPTH + 2)]
NT = STARTS[DEPTH + 1]          # 299593 nodes per tree
CG = C * NGEN                   # 128
NQ = 4                          # quarters per tree (core = tree x quarter)
LEAVES_Q = (K ** DEPTH) // NQ   # 65536 leaves per core
NP5 = LEAVES_Q // K             # 8192 level-5 parents per core
NP4 = NP5 // K                  # 1024 level-4 parents per core
CW = 512                        # chunk width (one PSUM bank of f32)
NCH = NP5 // CW                 # 16 chunks
SCALE = 128.0                   # fp8 range centering; cancels in beta
_SR_OFF = 256                   # byte offsets in the small packed table
TABB = 512


def _softmax64(x, axis):
    x = np.asarray(x, np.float64)
    e = np.exp(x - x.max(axis=axis, keepdims=True))
    return e / e.sum(axis=axis, keepdims=True)


def _build_tables(A, B, Pi, SP):
    """Small O(params) tables, f64 on host."""
    smA = _softmax64(A, 0)            # (C,C,K,G) over parent state i
    smB = _softmax64(B, 1)            # (C,M,G) over symbols
    smPi = _softmax64(Pi, 0)          # (C,K,G)
    smSP = _softmax64(SP, 0)          # (K,G)
    Mmat = smSP[:, None, None, :] * np.transpose(smA, (2, 0, 1, 3))  # [l,i,j,g]
    pb = smPi[:, :, None, :] * smB[:, None, :, :]      # (j, l, s, g)
    nuL = pb.sum(0)                                     # (l, s, g)
    betaLeaf = pb / nuL[None]
    llLeaf = np.log(nuL)                                # (l, s, g)
    T6 = np.einsum('lijg,jlsg->lsig', Mmat, betaLeaf)   # (l,s,i,g)
    T6f = (T6 * SCALE).reshape(K * MSYM, CG)            # rows (l,s), cols (i,g)
    Wl = np.zeros((K, CG, CG))
    ii = np.arange(C)
    for l in range(K):
        for g in range(NGEN):
            Wl[l, ii[:, None] * NGEN + g, ii[None, :] * NGEN + g] = Mmat[l, :, :, g].T
    Wt = np.concatenate([Wl[l] for l in range(K)], axis=1)   # [128, 1024]
    p = np.arange(CG)
    SR = (p[:, None] % NGEN == p[None, :] % NGEN).astype(np.float64)  # [128,128]
    BTcg = np.transpose(smB, (1, 0, 2)).reshape(MSYM, CG).T  # [(i,g), s]

    tabs = np.zeros((CG, TABB), np.uint8)
    tabs[:, 0:128] = T6f[:128].T.astype(F8).view(np.uint8)
    tabs[:, 128:256] = T6f[128:].T.astype(F8).view(np.uint8)
    tabs[:, _SR_OFF:_SR_OFF + 256] = SR.astype(BF16).view(np.uint8)
    return tabs, Wt.astype(BF16), Mmat, smB, BTcg, llLeaf


def _build_bass(repeat=1):
    import concourse.bass as bass
    import concourse.bacc as bacc
    import concourse.mybir as mybir
    from concourse import tile

    f32 = mybir.dt.float32
    bf16 = mybir.dt.bfloat16
    f8 = mybir.dt.float8e4
    u8 = mybir.dt.uint8
    Alu = mybir.AluOpType
    Act = mybir.ActivationFunctionType
    DR = mybir.MatmulPerfMode.DoubleRow

    nc = bacc.Bacc(None, target_bir_lowering=False)

    tabs_d = nc.dram_tensor('tabs', [CG, TABB], u8, kind='ExternalInput')
    wt_d = nc.dram_tensor('wt', [CG, 1024], bf16, kind='ExternalInput')
    oh_d = [nc.dram_tensor(f'oh{k}', [CG, 4096], f8, kind='ExternalInput')
            for k in range(4)]
    bx_d = [nc.dram_tensor(f'bx{k}', [CG, 2048], f8, kind='ExternalInput')
            for k in range(4)]
    tb4_d = [nc.dram_tensor(f'tb4_{h}', [CG, 256], f32, kind='ExternalOutput')
             for h in range(4)]
    ll5_d = nc.dram_tensor('ll5', [NGEN, NCH], f32, kind='ExternalOutput')

    with tile.TileContext(nc) as tc:
        with (
            tc.tile_pool(name='const', bufs=2) as constp,
            tc.tile_pool(name='oh', bufs=2) as ohp,
            tc.tile_pool(name='bx', bufs=2) as bxp,
            tc.tile_pool(name='bl', bufs=6) as blp,
            tc.tile_pool(name='beta', bufs=2) as betap,
            tc.tile_pool(name='ln', bufs=2) as lnp,
            tc.tile_pool(name='acc', bufs=2) as accp,
            tc.tile_pool(name='ps_tb', bufs=3, space='PSUM') as ps_tb,
            tc.tile_pool(name='ps_nu', bufs=3, space='PSUM') as ps_nu,
            tc.tile_pool(name='ps_w', bufs=1, space='PSUM') as ps_w,
        ):
            for rep in range(repeat):
                r = f'_{rep}' if repeat > 1 else ''
                tabs_t = constp.tile([CG, TABB], u8, name=f'tabs{r}', tag='tabs')
                wt_t = constp.tile([CG, 1024], bf16, name=f'wt{r}', tag='wt')
                oh_t = [ohp.tile([CG, 4096], f8, name=f'oh{k}{r}', tag=f'oh{k}')
                        for k in range(4)]
                bx_t = bxp.tile([CG, NP5], f8, name=f'bx{r}', tag='bx')
                beta5 = betap.tile([CG, NP5], bf16, name=f'b5{r}', tag='b5')
                llparts = accp.tile([NGEN, NCH], f32, name=f'llp{r}', tag='llp')
                tb4_sb = accp.tile([CG, NP4], f32, name=f'tb4{r}', tag='tb4')

                # prewarm the Ln activation table while input DMAs are in
                # flight, so the 1.3us table load is off the critical path
                warm = lnp.tile([NGEN, 1], f32, name=f'warm{r}', tag='warm')
                nc.gpsimd.memset(warm[:], 1.0)
                nc.scalar.activation(warm[:], warm[:], Act.Ln)

                # input DMAs spread across the three DMA-capable sequencers,
                # issued in first-need order (chunk 0's slices split out)
                def seg2(t_ap, dram, width, off=0):
                    src = bass.AP(dram[:].tensor, off,
                                  [[4096, CG], [2048, 2], [1, width]])
                    dst = bass.AP(t_ap.tensor, t_ap.offset + off,
                                  [t_ap.ap[0], [2048, 2], [1, width]])
                    return dst, src

                oh0 = oh_t[0][:]
                d, s = seg2(oh0, oh_d[0], CW)
                nc.scalar.dma_start(d, s)
                nc.gpsimd.dma_start(bx_t[:, 0:CW], bx_d[0][:, 0:CW])
                nc.sync.dma_start(tabs_t[:], tabs_d[:])
                d, s = seg2(oh0, oh_d[0], 2048 - CW, off=CW)
                nc.scalar.dma_start(d, s)
                nc.gpsimd.dma_start(bx_t[:, CW:2048], bx_d[0][:, CW:2048])
                nc.sync.dma_start(oh_t[1][:], oh_d[1][:])
                nc.scalar.dma_start(bx_t[:, 2048:4096], bx_d[1][:])
                nc.sync.dma_start(oh_t[2][:], oh_d[2][:])
                nc.gpsimd.dma_start(bx_t[:, 4096:6144], bx_d[2][:])
                nc.sync.dma_start(oh_t[3][:], oh_d[3][:])
                nc.scalar.dma_start(bx_t[:, 6144:NP5], bx_d[3][:])
                nc.sync.dma_start(wt_t[:], wt_d[:])

                # fp8 DoubleRow weights: k-tile 0 = T6a, k-tile 1 = T6b
                t6ab = tabs_t[:, 0:256].bitcast(f8).rearrange(
                    'p (k q) -> p k q', k=2)
                SR = tabs_t[:, _SR_OFF:_SR_OFF + 256].bitcast(bf16)

                bview = beta5[:].rearrange('p (u l) -> p u l', l=K)
                bl_tiles = {}

                def emit_head(c):
                    k, j = c // 4, c % 4
                    tb_ps = ps_tb.tile([CG, CW], f32, name=f'tb{r}', tag='tb')
                    t = oh_t[k][:]
                    rhs = bass.AP(t.tensor, t.offset + j * CW,
                                  [t.ap[0], [2048, 2], [1, CW]])
                    nc.tensor.matmul(tb_ps[:], t6ab, rhs, start=True, stop=True,
                                     perf_mode=DR)
                    bl_t = blp.tile([CG, CW], bf16, name=f'bl{r}', tag='bl')
                    nc.vector.tensor_mul(bl_t[:], tb_ps[:],
                                         bx_t[:, c * CW:(c + 1) * CW])
                    bl_tiles[c] = bl_t

                def emit_tail(c):
                    bl_t = bl_tiles.pop(c)
                    nu_ps = ps_nu.tile([CG, CW], f32, name=f'nu{r}', tag='nu')
                    nc.tensor.matmul(nu_ps[:], SR, bl_t[:], start=True, stop=True)
                    ln_t = lnp.tile([NGEN, CW], f32, name=f'ln{r}', tag='ln')
                    nc.scalar.activation(ln_t[:], nu_ps[0:NGEN, :], Act.Ln,
                                         accum_out=llparts[:, c:c + 1])
                    # GPSIMD/Pool cannot touch PSUM and DVE has no divide ISA
                    # op, so renorm = DVE reciprocal (PSUM -> SBUF bf16, the
                    # SR matmul already broadcast nu to all 128 partitions)
                    # followed by an all-SBUF multiply on Pool
                    r_t = blp.tile([CG, CW], bf16, name=f'rc{r}', tag='rc')
                    with nc.allow_low_precision(reason='bf16 recip, validated vs f64 host'):
                        nc.vector.reciprocal(r_t[:], nu_ps[:])
                    nc.gpsimd.tensor_mul(beta5[:, c * CW:(c + 1) * CW],
                                         bl_t[:], r_t[:])

                def emit_w(ph):
                    # level 5 -> 4 over parents [128*ph, 128*(ph+1)), i.e.
                    # beta5 chunks 2ph..2ph+1; drain on DVE; ship per quarter
                    w_ps = ps_w.tile([CG, 128], f32, name=f'w{ph}{r}',
                                     tag=f'w{ph % 2}')
                    for l in range(K):
                        nc.tensor.matmul(w_ps[:], wt_t[:, 128 * l:128 * (l + 1)],
                                         bview[:, ph * 128:(ph + 1) * 128, l],
                                         start=(l == 0), stop=(l == K - 1))
                    piece = tb4_sb[:, ph * 128:(ph + 1) * 128]
                    nc.vector.tensor_scalar_add(piece, w_ps[:], 0.0)
                    if ph % 2 == 1:
                        qd = ph // 2
                        nc.scalar.dma_start(
                            tb4_d[qd][:], tb4_sb[:, qd * 256:(qd + 1) * 256])

                # software-pipelined level 6 -> 5 (tail lags head by LAG);
                # W phase ph consumes chunks 2ph..2ph+1
                LAG = 3
                for c in range(NCH):
                    emit_head(c)
                    if c >= LAG:
                        emit_tail(c - LAG)
                        if (c - LAG) % 2 == 1:
                            emit_w((c - LAG) // 2)
                for c in range(NCH - LAG, NCH):
                    emit_tail(c)
                    if c % 2 == 1:
                        emit_w(c // 2)

                nc.sync.dma_start(ll5_d[:], llparts[:])
    if not nc.is_finalized():
        nc.finalize()
    return nc


_BASS_CACHE = {}


def _get_bass(repeat=1):
    if repeat not in _BASS_CACHE:
        _BASS_CACHE[repeat] = _build_bass(repeat)
    return _BASS_CACHE[repeat]


def kernel(**inputs):
    from concourse.bass_utils import run_bass_kernel_spmd

    A = np.asarray(inputs['A']); B = np.asarray(inputs['B'])
    Pi = np.asarray(inputs['Pi']); SP = np.asarray(inputs['SP'])
    x = np.asarray(inputs['x'])

    tabs, Wt_bf, Mmat, smB, BTcg, llLeaf = _build_tables(A, B, Pi, SP)
    BT_f8 = (BTcg * SCALE).astype(F8)

    in_maps = []
    for t in range(NTREE):
        base = t * NT
        for q in range(NQ):
            s6 = base + STARTS[6] + q * LEAVES_Q
            xs_t = x[s6: s6 + LEAVES_Q].reshape(NP5, K).T      # [8, 8192]
            s5 = base + STARTS[5] + q * NP5
            x5 = x[s5: s5 + NP5]
            oh = np.zeros((CG, 2 * NP5), F8)
            cols = np.arange(NP5)
            one = F8(1.0)
            for l in range(4):
                oh[l * MSYM + xs_t[l], cols] = one
                oh[l * MSYM + xs_t[4 + l], NP5 + cols] = one
            bx5 = BT_f8[:, x5]                                  # [128, 8192]
            m = {'tabs': tabs, 'wt': Wt_bf}
            for k in range(4):
                m[f'oh{k}'] = np.ascontiguousarray(
                    np.concatenate([oh[:, k * 2048:(k + 1) * 2048],
                                    oh[:, NP5 + k * 2048:NP5 + (k + 1) * 2048]], 1))
                m[f'bx{k}'] = np.ascontiguousarray(bx5[:, k * 2048:(k + 1) * 2048])
            in_maps.append(m)

    nc = _get_bass()
    global _LAST_IN_MAPS
    _LAST_IN_MAPS = in_maps
    res = run_bass_kernel_spmd(nc, in_maps, core_ids=list(range(8)))
    results = res.results

    out = np.zeros((NTREE, NGEN), np.float64)
    lnscale = 2.0 * np.log(SCALE)
    for t in range(NTREE):
        base = t * NT
        # level-5 log-nus accumulated on device (fp8 scale correction here)
        for q in range(NQ):
            r = results[t * NQ + q]
            out[t] += r['ll5'].astype(np.float64).sum(1) - NP5 * lnscale

        # leaf log-nus: histogram x log-table, exact in f64
        xs = x[base + STARTS[6]: base + STARTS[6] + K ** DEPTH]
        idx = (np.arange(K ** DEPTH) % K) * MSYM + xs
        counts = np.bincount(idx, minlength=K * MSYM).astype(np.float64)
        out[t] += counts @ llLeaf.reshape(K * MSYM, NGEN)

        # levels 4..1 + root on host from device tb4, f64 softmax math
        tb4 = np.concatenate(
            [results[t * NQ + q][f'tb4_{h}'].astype(np.float64)
             for q in range(NQ) for h in range(4)],
            axis=1)                                             # [128, 4096]
        tb = tb4.T.reshape(K ** 4, C, NGEN)                     # (u, i, g)
        for d in range(4, -1, -1):
            n_d = K ** d
            s_d = base + STARTS[d]
            x_d = x[s_d: s_d + n_d]
            bl = tb * np.transpose(smB[:, x_d], (1, 0, 2))      # (u, C, G)
            nu = bl.sum(1)
            out[t] += np.log(nu).sum(0)
            if d == 0:
                break
            beta = bl / nu[:, None]
            bch = beta.reshape(n_d // K, K, C, NGEN)
            tb = np.einsum('uljg,lijg->uig', bch, Mmat)
    return out.astype(np.float32)


# revision 5
# speedup vs baseline: 3.4560x; 1.9695x over previous
"""Bottom-Up Hidden Tree Markov Model upward pass on 8 Trainium2 NeuronCores.

Problem: complete 8-ary forest (2 trees x 299593 nodes, depth 6), C=8 hidden
states, 32 symbols, 16 independent generative models. Output: per-tree
log-likelihood (2, 16).

Sharding: core = (tree, quarter-of-tree). Each core runs the upward pass over
its quarter's two big levels (98.4% of all nodes): 65536 leaves -> 8192
level-5 betas (one fp8 DoubleRow matmul per 512 parents against the collapsed
leaf table T6, contracting all 256 (position, symbol) one-hot rows at once)
and 8192 -> 1024 level-4 t_betas (block-diagonal W matmuls). The host applies
the exact softmax recursion to the tiny tree top (levels 4..1 + root, 1.6% of
nodes) and the leaf log-nu histogram, both in float64.

Device pipeline per 512-parent chunk (engines balanced, dispatch-minimal):
  PE:   tb   = [T6a;T6b]^T [ohA;ohB]           (fp8 DoubleRow, PSUM f32)
  DVE:  bl   = tb * bx                          (bf16 out)
  PE:   nu_b = SR^T bl                          (per-g sums broadcast to all
                                                 128 partitions in one matmul)
  Act:  Ln(nu) with free-dim accumulation       -> per-chunk log-nu partials
  Pool: beta = bl / nu_b                        (divide, bf16 out)
Partition packing everywhere: p = i*16 + g  (i = hidden state, g = generator).

T6 and bx are carried in fp8_e4m3 scaled by 128 (one-hots exact, ~2^-4
relative quantization); the uniform 128^2 scale cancels in beta and is
subtracted from the accumulated log-nus on the host.
"""
import sys

import numpy as np

if '/opt/trn_rl_repo' not in sys.path:
    sys.path.insert(0, '/opt/trn_rl_repo')

import ml_dtypes

BF16 = ml_dtypes.bfloat16
F8 = ml_dtypes.float8_e4m3

K, DEPTH, NTREE, C, MSYM, NGEN = 8, 6, 2, 8, 32, 16
STARTS = [(K ** d - 1) // (K - 1) for d in range(DEPTH + 2)]
NT = STARTS[DEPTH + 1]          # 299593 nodes per tree
CG = C * NGEN                   # 128
NQ = 4                          # quarters per tree (core = tree x quarter)
LEAVES_Q = (K ** DEPTH) // NQ   # 65536 leaves per core
NP5 = LEAVES_Q // K             # 8192 level-5 parents per core
NP4 = NP5 // K                  # 1024 level-4 parents per core
CW = 512                        # chunk width (one PSUM bank of f32)
NCH = NP5 // CW                 # 16 chunks
SCALE = 128.0                   # fp8 range centering; cancels in beta
_SR_OFF = 256                   # byte offsets in the small packed table
TABB = 512


def _softmax64(x, axis):
    x = np.asarray(x, np.float64)
    e = np.exp(x - x.max(axis=axis, keepdims=True))
    return e / e.sum(axis=axis, keepdims=True)


def _build_tables(A, B, Pi, SP):
    """Small O(params) tables, f64 on host."""
    smA = _softmax64(A, 0)            # (C,C,K,G) over parent state i
    smB = _softmax64(B, 1)            # (C,M,G) over symbols
    smPi = _softmax64(Pi, 0)          # (C,K,G)
    smSP = _softmax64(SP, 0)          # (K,G)
    Mmat = smSP[:, None, None, :] * np.transpose(smA, (2, 0, 1, 3))  # [l,i,j,g]
    pb = smPi[:, :, None, :] * smB[:, None, :, :]      # (j, l, s, g)
    nuL = pb.sum(0)                                     # (l, s, g)
    betaLeaf = pb / nuL[None]
    llLeaf = np.log(nuL)                                # (l, s, g)
    T6 = np.einsum('lijg,jlsg->lsig', Mmat, betaLeaf)   # (l,s,i,g)
    T6f = (T6 * SCALE).reshape(K * MSYM, CG)            # rows (l,s), cols (i,g)
    Wl = np.zeros((K, CG, CG))
    ii = np.arange(C)
    for l in range(K):
        for g in range(NGEN):
            Wl[l, ii[:, None] * NGEN + g, ii[None, :] * NGEN + g] = Mmat[l, :, :, g].T
    Wt = np.concatenate([Wl[l] for l in range(K)], axis=1)   # [128, 1024]
    p = np.arange(CG)
    SR = (p[:, None] % NGEN == p[None, :] % NGEN).astype(np.float64)  # [128,128]
    BTcg = np.transpose(smB, (1, 0, 2)).reshape(MSYM, CG).T  # [(i,g), s]

    tabs = np.zeros((CG, TABB), np.uint8)
    tabs[:, 0:128] = T6f[:128].T.astype(F8).view(np.uint8)
    tabs[:, 128:256] = T6f[128:].T.astype(F8).view(np.uint8)
    tabs[:, _SR_OFF:_SR_OFF + 256] = SR.astype(BF16).view(np.uint8)
    return tabs, Wt.astype(BF16), Mmat, smB, BTcg, llLeaf


def _build_bass(repeat=1):
    import concourse.bass as bass
    import concourse.bacc as bacc
    import concourse.mybir as mybir
    from concourse import tile

    f32 = mybir.dt.float32
    bf16 = mybir.dt.bfloat16
    f8 = mybir.dt.float8e4
    u8 = mybir.dt.uint8
    Alu = mybir.AluOpType
    Act = mybir.ActivationFunctionType
    DR = mybir.MatmulPerfMode.DoubleRow

    nc = bacc.Bacc(None, target_bir_lowering=False)

    tabs_d = nc.dram_tensor('tabs', [CG, TABB], u8, kind='ExternalInput')
    wt_d = nc.dram_tensor('wt', [CG, 1024], bf16, kind='ExternalInput')
    oh_d = [nc.dram_tensor(f'oh{k}', [CG, 4096], f8, kind='ExternalInput')
            for k in range(4)]
    bx_d = [nc.dram_tensor(f'bx{k}', [CG, 2048], f8, kind='ExternalInput')
            for k in range(4)]
    tb4_d = [nc.dram_tensor(f'tb4_{h}', [CG, 256], f32, kind='ExternalOutput')
             for h in range(4)]
    nu5_d = nc.dram_tensor('nu5', [NGEN, NP5], f32, kind='ExternalOutput')

    with tile.TileContext(nc) as tc:
        with (
            tc.tile_pool(name='const', bufs=2) as constp,
            tc.tile_pool(name='oh', bufs=2) as ohp,
            tc.tile_pool(name='bx', bufs=2) as bxp,
            tc.tile_pool(name='bl', bufs=6) as blp,
            tc.tile_pool(name='beta', bufs=2) as betap,
            tc.tile_pool(name='acc', bufs=2) as accp,
            tc.tile_pool(name='ps_tb', bufs=3, space='PSUM') as ps_tb,
            tc.tile_pool(name='ps_nu', bufs=3, space='PSUM') as ps_nu,
            tc.tile_pool(name='ps_w', bufs=1, space='PSUM') as ps_w,
        ):
            for rep in range(repeat):
                r = f'_{rep}' if repeat > 1 else ''
                tabs_t = constp.tile([CG, TABB], u8, name=f'tabs{r}', tag='tabs')
                wt_t = constp.tile([CG, 1024], bf16, name=f'wt{r}', tag='wt')
                oh_t = [ohp.tile([CG, 4096], f8, name=f'oh{k}{r}', tag=f'oh{k}')
                        for k in range(4)]
                bx_t = bxp.tile([CG, NP5], f8, name=f'bx{r}', tag='bx')
                beta5 = betap.tile([CG, NP5], bf16, name=f'b5{r}', tag='b5')
                nu5_sb = accp.tile([NGEN, NP5], f32, name=f'nu5{r}', tag='nu5')
                tb4_sb = accp.tile([CG, NP4], f32, name=f'tb4{r}', tag='tb4')

                # input DMAs spread across the three DMA-capable sequencers,
                # issued in first-need order (chunk 0's slices split out)
                def seg2(t_ap, dram, width, off=0):
                    src = bass.AP(dram[:].tensor, off,
                                  [[4096, CG], [2048, 2], [1, width]])
                    dst = bass.AP(t_ap.tensor, t_ap.offset + off,
                                  [t_ap.ap[0], [2048, 2], [1, width]])
                    return dst, src

                oh0 = oh_t[0][:]
                d, s = seg2(oh0, oh_d[0], CW)
                nc.scalar.dma_start(d, s)
                nc.gpsimd.dma_start(bx_t[:, 0:CW], bx_d[0][:, 0:CW])
                nc.sync.dma_start(tabs_t[:], tabs_d[:])
                d, s = seg2(oh0, oh_d[0], 2048 - CW, off=CW)
                nc.scalar.dma_start(d, s)
                nc.gpsimd.dma_start(bx_t[:, CW:2048], bx_d[0][:, CW:2048])
                nc.sync.dma_start(oh_t[1][:], oh_d[1][:])
                nc.scalar.dma_start(bx_t[:, 2048:4096], bx_d[1][:])
                nc.sync.dma_start(oh_t[2][:], oh_d[2][:])
                nc.gpsimd.dma_start(bx_t[:, 4096:6144], bx_d[2][:])
                nc.sync.dma_start(oh_t[3][:], oh_d[3][:])
                nc.scalar.dma_start(bx_t[:, 6144:NP5], bx_d[3][:])
                nc.sync.dma_start(wt_t[:], wt_d[:])

                # fp8 DoubleRow weights: k-tile 0 = T6a, k-tile 1 = T6b
                t6ab = tabs_t[:, 0:256].bitcast(f8).rearrange(
                    'p (k q) -> p k q', k=2)
                SR = tabs_t[:, _SR_OFF:_SR_OFF + 256].bitcast(bf16)

                bview = beta5[:].rearrange('p (u l) -> p u l', l=K)
                bl_tiles = {}

                def emit_head(c):
                    k, j = c // 4, c % 4
                    tb_ps = ps_tb.tile([CG, CW], f32, name=f'tb{r}', tag='tb')
                    t = oh_t[k][:]
                    rhs = bass.AP(t.tensor, t.offset + j * CW,
                                  [t.ap[0], [2048, 2], [1, CW]])
                    nc.tensor.matmul(tb_ps[:], t6ab, rhs, start=True, stop=True,
                                     perf_mode=DR)
                    bl_t = blp.tile([CG, CW], bf16, name=f'bl{r}', tag='bl')
                    nc.vector.tensor_mul(bl_t[:], tb_ps[:],
                                         bx_t[:, c * CW:(c + 1) * CW])
                    bl_tiles[c] = bl_t

                def emit_tail(c):
                    bl_t = bl_tiles.pop(c)
                    nu_ps = ps_nu.tile([CG, CW], f32, name=f'nu{r}', tag='nu')
                    nc.tensor.matmul(nu_ps[:], SR, bl_t[:], start=True, stop=True)
                    nc.scalar.copy(nu5_sb[:, c * CW:(c + 1) * CW],
                                   nu_ps[0:NGEN, :])
                    # GPSIMD/Pool cannot touch PSUM and DVE has no divide ISA
                    # op, so renorm = DVE reciprocal (PSUM -> SBUF bf16, the
                    # SR matmul already broadcast nu to all 128 partitions)
                    # followed by an all-SBUF multiply on Pool
                    r_t = blp.tile([CG, CW], bf16, name=f'rc{r}', tag='rc')
                    with nc.allow_low_precision(reason='bf16 recip, validated vs f64 host'):
                        nc.vector.reciprocal(r_t[:], nu_ps[:])
                    nc.gpsimd.tensor_mul(beta5[:, c * CW:(c + 1) * CW],
                                         bl_t[:], r_t[:])

                def emit_w(ph):
                    # level 5 -> 4 over parents [128*ph, 128*(ph+1)), i.e.
                    # beta5 chunks 2ph..2ph+1; drain on DVE; ship per quarter
                    w_ps = ps_w.tile([CG, 128], f32, name=f'w{ph}{r}',
                                     tag=f'w{ph % 2}')
                    for l in range(K):
                        nc.tensor.matmul(w_ps[:], wt_t[:, 128 * l:128 * (l + 1)],
                                         bview[:, ph * 128:(ph + 1) * 128, l],
                                         start=(l == 0), stop=(l == K - 1))
                    nc.scalar.copy(tb4_sb[:, ph * 128:(ph + 1) * 128], w_ps[:])
                    if ph % 2 == 1:
                        qd = ph // 2
                        nc.scalar.dma_start(
                            tb4_d[qd][:], tb4_sb[:, qd * 256:(qd + 1) * 256])

                # software-pipelined level 6 -> 5 (tail lags head by LAG);
                # W phase ph consumes chunks 2ph..2ph+1
                LAG = 3
                for c in range(NCH):
                    emit_head(c)
                    if c >= LAG:
                        emit_tail(c - LAG)
                        if (c - LAG) % 2 == 1:
                            emit_w((c - LAG) // 2)
                for c in range(NCH - LAG, NCH):
                    emit_tail(c)
                    if c % 2 == 1:
                        emit_w(c // 2)

                nc.sync.dma_start(nu5_d[:], nu5_sb[:])
    if not nc.is_finalized():
        nc.finalize()
    return nc


_BASS_CACHE = {}


def _get_bass(repeat=1):
    if repeat not in _BASS_CACHE:
        _BASS_CACHE[repeat] = _build_bass(repeat)
    return _BASS_CACHE[repeat]


def kernel(**inputs):
    from concourse.bass_utils import run_bass_kernel_spmd

    A = np.asarray(inputs['A']); B = np.asarray(inputs['B'])
    Pi = np.asarray(inputs['Pi']); SP = np.asarray(inputs['SP'])
    x = np.asarray(inputs['x'])

    tabs, Wt_bf, Mmat, smB, BTcg, llLeaf = _build_tables(A, B, Pi, SP)
    BT_f8 = (BTcg * SCALE).astype(F8)

    in_maps = []
    for t in range(NTREE):
        base = t * NT
        for q in range(NQ):
            s6 = base + STARTS[6] + q * LEAVES_Q
            xs_t = x[s6: s6 + LEAVES_Q].reshape(NP5, K).T      # [8, 8192]
            s5 = base + STARTS[5] + q * NP5
            x5 = x[s5: s5 + NP5]
            oh = np.zeros((CG, 2 * NP5), F8)
            cols = np.arange(NP5)
            one = F8(1.0)
            for l in range(4):
                oh[l * MSYM + xs_t[l], cols] = one
                oh[l * MSYM + xs_t[4 + l], NP5 + cols] = one
            bx5 = BT_f8[:, x5]                                  # [128, 8192]
            m = {'tabs': tabs, 'wt': Wt_bf}
            for k in range(4):
                m[f'oh{k}'] = np.ascontiguousarray(
                    np.concatenate([oh[:, k * 2048:(k + 1) * 2048],
                                    oh[:, NP5 + k * 2048:NP5 + (k + 1) * 2048]], 1))
                m[f'bx{k}'] = np.ascontiguousarray(bx5[:, k * 2048:(k + 1) * 2048])
            in_maps.append(m)

    nc = _get_bass()
    global _LAST_IN_MAPS
    _LAST_IN_MAPS = in_maps
    res = run_bass_kernel_spmd(nc, in_maps, core_ids=list(range(8)))
    results = res.results

    out = np.zeros((NTREE, NGEN), np.float64)
    lnscale = 2.0 * np.log(SCALE)
    for t in range(NTREE):
        base = t * NT
        # level-5 log-nus from exact device f32 nus, logs in f64 on host
        # (fp8 scale correction is exact: nu is scaled by SCALE^2)
        for q in range(NQ):
            r = results[t * NQ + q]
            out[t] += np.log(r['nu5'].astype(np.float64)).sum(1) - NP5 * lnscale

        # leaf log-nus: histogram x log-table, exact in f64
        xs = x[base + STARTS[6]: base + STARTS[6] + K ** DEPTH]
        idx = (np.arange(K ** DEPTH) % K) * MSYM + xs
        counts = np.bincount(idx, minlength=K * MSYM).astype(np.float64)
        out[t] += counts @ llLeaf.reshape(K * MSYM, NGEN)

        # levels 4..1 + root on host from device tb4, f64 softmax math
        tb4 = np.concatenate(
            [results[t * NQ + q][f'tb4_{h}'].astype(np.float64)
             for q in range(NQ) for h in range(4)],
            axis=1)                                             # [128, 4096]
        tb = tb4.T.reshape(K ** 4, C, NGEN)                     # (u, i, g)
        for d in range(4, -1, -1):
            n_d = K ** d
            s_d = base + STARTS[d]
            x_d = x[s_d: s_d + n_d]
            bl = tb * np.transpose(smB[:, x_d], (1, 0, 2))      # (u, C, G)
            nu = bl.sum(1)
            out[t] += np.log(nu).sum(0)
            if d == 0:
                break
            beta = bl / nu[:, None]
            bch = beta.reshape(n_d // K, K, C, NGEN)
            tb = np.einsum('uljg,lijg->uig', bch, Mmat)
    return out.astype(np.float32)
